# revision 17
# baseline (speedup 1.0000x reference)
"""Trainium2 Bass kernel for nn_CrossAttention (B=4, NQ=512, NKV=4096, H=12, D=64).

Sharding: 8 cores = 4 batches x 2 head-groups (6 heads each). Each core computes
its (batch, head-group) slice of cross-attention and a partial output projection
(contribution of its 384 attn channels to all 768 output channels). Host sums
the two head-group partials per batch and adds bproj.

Key structure (cost model charges a matmul by its output free size only):
  - attn@V runs "flipped": out[q(128 part), d+1(65 free)] accumulated over kt,
    with a ones column in V giving the softmax denominator in col 64. This
    uses all 128 output partitions (vs 65 in the naive orientation) and makes
    normalization a per-partition scalar multiply.
  - The normalized [q, 2*64] tile is transposed back to [ac, q] with the DMA
    xbar (dma_start_transpose), not the PE.
  - Output projection runs as out[q, oc] with Wproj as the natural rhs.
  - exp runs on Activation (~100us total) while PE (~131us) is kept fed by
    interleaving K/V projection matmuls into the attention kt loops.
Engines: PE matmuls; Act exp; DVE rope muls/adds + norms + psum copies;
GpSimd perm DMAs + V copies; SP input/transpose/output DMAs.
"""

import numpy as np
import ml_dtypes

import concourse.bass as bass
from concourse import bacc
import concourse.mybir as mybir
import concourse.tile as tile
from concourse.bass_utils import run_bass_kernel_spmd

BF16 = ml_dtypes.bfloat16

B, NQ, NKV = 4, 512, 4096
LATENT = 768
H, D = 12, 64
G = 2                  # head groups (cores per batch)
HPG = H // G           # heads per group = 6
DG = HPG * D           # 384 attn channels per group
P = 128
CSUB = LATENT // P     # 6 contraction subtiles
NKT = NKV // P         # 32 k-tiles
NCH = NKV // 512       # 8 512-col data chunks
PAIRS = HPG // 2       # 3 head pairs
QB = NQ // P           # 4 q blocks

FP32 = mybir.dt.float32
BF16_DT = mybir.dt.bfloat16
AOP = mybir.AluOpType
EXP = mybir.ActivationFunctionType.Exp


def _build_program():
    nc = bacc.Bacc()

    def din(name, shape):
        return nc.dram_tensor(name, shape, BF16_DT, kind="ExternalInput")

    latentT = din("latentT", [LATENT, NQ])
    dataT = din("dataT", [LATENT, NKV])
    wq = din("wq", [LATENT, DG])        # pre-scaled by D^-0.5
    wk = din("wk", [LATENT, DG])
    wv = din("wv", [LATENT, DG])
    wproj = din("wproj", [DG, LATENT])
    ropeq = din("ropeq", [P, 2, NQ])    # [128, (cos|sin), n]; 64 rows x2, sin sign-folded
    ropek = din("ropek", [P, 2, NKV])
    ident = din("ident", [P, P])
    out_d = nc.dram_tensor("out", [NQ, LATENT], BF16_DT, kind="ExternalOutput")

    lat_v = latentT.rearrange("(o p) q -> p o q", p=P)
    data_v = dataT.rearrange("(o p) k -> p o k", p=P)
    wq_v = wq.rearrange("(o p) n -> p o n", p=P)
    wk_v = wk.rearrange("(o p) n -> p o n", p=P)
    wv_v = wv.rearrange("(o p) n -> p o n", p=P)
    wproj_v = wproj.rearrange("(o p) n -> p o n", p=P)   # [128, 3, 768]

    with tile.TileContext(nc) as tc:
        with (
            tc.tile_pool(name="singles", bufs=1) as singles,
            tc.tile_pool(name="ropep", bufs=2) as ropep,
            tc.tile_pool(name="ep", bufs=3) as ep,
            tc.tile_pool(name="np_pool", bufs=2) as np_pool,
            tc.tile_pool(name="pp", bufs=2, space="PSUM") as pp,
            tc.tile_pool(name="pss", bufs=2, space="PSUM") as pss,
            tc.tile_pool(name="psa", bufs=2, space="PSUM") as psa,
        ):
            # ---- resident SBUF + input DMAs in need order (SP stream) ------
            lat_sb = singles.tile([P, CSUB, NQ], BF16_DT)
            wq_sb = singles.tile([P, CSUB, DG], BF16_DT)
            nc.sync.dma_start(lat_sb[:, 0:3, :], lat_v[:, 0:3, :])
            nc.sync.dma_start(wq_sb[:, 0:3, :], wq_v[:, 0:3, :])
            nc.sync.dma_start(lat_sb[:, 3:6, :], lat_v[:, 3:6, :])
            nc.sync.dma_start(wq_sb[:, 3:6, :], wq_v[:, 3:6, :])
            ropeq_sb = singles.tile([P, 2, NQ], BF16_DT)
            nc.sync.dma_start(ropeq_sb, ropeq[:])
            cosq_sb = ropeq_sb[:, 0, :]
            sinq_sb = ropeq_sb[:, 1, :]
            wk_sb = singles.tile([P, CSUB, DG], BF16_DT)
            nc.sync.dma_start(wk_sb, wk_v)

            data_sb = singles.tile([P, CSUB, NKV], BF16_DT)
            ropek_sb = singles.tile([P, 2, NKV], BF16_DT)
            cosk_sb = ropek_sb[:, 0, :]
            sink_sb = ropek_sb[:, 1, :]

            def dma_data(c):
                sl = slice(c * 512, (c + 1) * 512)
                nc.sync.dma_start(data_sb[:, :, sl], data_v[:, :, sl])

            def dma_rope_k(q):
                sl = slice(q * 1024, (q + 1) * 1024)
                nc.sync.dma_start(ropek_sb[:, :, sl], ropek[:, :, sl])

            dma_data(0)
            wv_sb = singles.tile([P, CSUB, DG], BF16_DT)
            nc.sync.dma_start(wv_sb, wv_v)
            dma_rope_k(0)
            dma_data(1)
            dma_data(2)
            dma_data(3)
            dma_rope_k(1)
            dma_data(4)
            dma_data(5)
            dma_rope_k(2)
            dma_data(6)
            dma_data(7)
            dma_rope_k(3)
            wproj_sb = singles.tile([P, PAIRS, LATENT], BF16_DT)
            nc.sync.dma_start(wproj_sb, wproj_v)
            ident_sb = singles.tile([P, P], BF16_DT)
            nc.sync.dma_start(ident_sb, ident[:])

            qt_sb = singles.tile([P, PAIRS, NQ], BF16_DT)      # roped Q^T
            kt_sb = [
                singles.tile([P, NKV], BF16_DT, name=f"kt{j}")
                for j in range(PAIRS)
            ]
            cat_sb = [
                singles.tile([P, NQ], BF16_DT, name=f"cat{j}")
                for j in range(PAIRS)
            ]
            v_sb = singles.tile([P, NKT, HPG, D + 1], BF16_DT)
            nc.gpsimd.memset(v_sb[:, :, :, D : D + 1], 1.0)

            # ---- helpers ---------------------------------------------------
            def perm_dma(dst, src, eng=None):
                """dst = src with 32-row halves swapped within each 64-row
                block (the rot-half partition shuffle). eng=nc.scalar uses the
                Activation HWDGE (fast, for the pre-phase while Act is idle);
                default GpSimd SWDGE keeps Act free for exp mid-flight."""
                eng = eng or nc.gpsimd
                for blk in range(2):
                    b0 = blk * 64
                    eng.dma_start(dst[b0 : b0 + 32, :], src[b0 + 32 : b0 + 64, :])
                    eng.dma_start(dst[b0 + 32 : b0 + 64, :], src[b0 : b0 + 32, :])

            # ---- Q projection + rope ---------------------------------------
            qraw = singles.tile([P, PAIRS * NQ], BF16_DT)
            for j in range(PAIRS):
                ps = pp.tile([P, NQ], FP32, tag="pp", name="ps_q")
                for cs in range(CSUB):
                    nc.tensor.matmul(
                        ps,
                        lhsT=wq_sb[:, cs, j * P : (j + 1) * P],
                        rhs=lat_sb[:, cs, :],
                        start=(cs == 0),
                        stop=(cs == CSUB - 1),
                    )
                nc.vector.tensor_copy(qraw[:, j * NQ : (j + 1) * NQ], ps)
            qperm = singles.tile([P, PAIRS * NQ], BF16_DT)
            perm_dma(qperm, qraw, eng=nc.scalar)
            for j in range(PAIRS):
                sl = slice(j * NQ, (j + 1) * NQ)
                nc.vector.tensor_tensor(qraw[:, sl], qraw[:, sl], cosq_sb, AOP.mult)
                nc.vector.tensor_tensor(qperm[:, sl], qperm[:, sl], sinq_sb, AOP.mult)
                nc.vector.tensor_tensor(qt_sb[:, j, :], qraw[:, sl], qperm[:, sl], AOP.add)

            # ---- K projection (per 512-col chunk) + rope (per 1024 quarter)
            kraw = {}

            def kp_chunk(j, ch):
                sl = slice(ch * 512, (ch + 1) * 512)
                ps = pp.tile([P, 512], FP32, tag="pp", name="ps_k")
                for cs in range(CSUB):
                    nc.tensor.matmul(
                        ps,
                        lhsT=wk_sb[:, cs, j * P : (j + 1) * P],
                        rhs=data_sb[:, cs, sl],
                        start=(cs == 0),
                        stop=(cs == CSUB - 1),
                    )
                quarter = ch // 2
                if ch % 2 == 0:
                    # bufs=4: j=0 and j=1 kraw lifetimes overlap out of
                    # rotation order in att(0) (j=1 chunks land early but
                    # rope late); 2 slots would deadlock the DVE stream.
                    kraw[(j, quarter)] = ropep.tile(
                        [P, 1024], BF16_DT, tag="kraw", bufs=4, name=f"kraw{j}_{quarter}"
                    )
                c2 = ch % 2
                nc.vector.tensor_copy(kraw[(j, quarter)][:, c2 * 512 : (c2 + 1) * 512], ps)

            def kp_rope(j, quarter, dma_eng=None, mul_eng=None):
                """rope for kt_sb[j] cols [1024q, 1024(q+1)). mul_eng=nc.gpsimd
                moves the combine off DVE (used at phase ends where DVE
                backlog would delay the norms)."""
                mul = mul_eng or nc.vector
                raw = kraw.pop((j, quarter))
                perm = ropep.tile([P, 1024], BF16_DT, tag="kperm", name=f"kperm{j}_{quarter}")
                perm_dma(perm, raw, eng=dma_eng)
                sl = slice(quarter * 1024, (quarter + 1) * 1024)
                mul.tensor_tensor(raw, raw, cosk_sb[:, sl], AOP.mult)
                mul.tensor_tensor(perm, perm, sink_sb[:, sl], AOP.mult)
                mul.tensor_tensor(kt_sb[j][:, sl], raw, perm, AOP.add)

            # ---- V projection for head pair pj, one k-tile -----------------
            # copy_eng: "act" while Activation has slack (pre/att0), else DVE
            def vp(pj, kt, copy_eng="dve"):
                ps = pp.tile([P, P], FP32, tag="pp", name="ps_v")
                for cs in range(CSUB):
                    nc.tensor.matmul(
                        ps,
                        lhsT=data_sb[:, cs, kt * P : (kt + 1) * P],
                        rhs=wv_sb[:, cs, pj * P : (pj + 1) * P],
                        start=(cs == 0),
                        stop=(cs == CSUB - 1),
                    )
                dst = v_sb[:, kt, 2 * pj : 2 * pj + 2, 0:D]
                src = ps.rearrange("p (h d) -> p h d", h=2)
                if copy_eng == "act":
                    nc.scalar.copy(dst, src)
                else:
                    nc.vector.tensor_copy(dst, src)

            # ---- attention: scores+exp for kt, attn@V one iter behind ------
            po = {}
            e_tiles = {}

            def att_scores(j, kt):
                ps_s = pss.tile([P, 2 * NQ], FP32, tag="ss", name="ps_s")
                nc.tensor.matmul(
                    ps_s[:, 0:NQ],
                    lhsT=kt_sb[j][0:64, kt * P : (kt + 1) * P],
                    rhs=qt_sb[0:64, j, :],
                    start=True,
                    stop=True,
                )
                nc.tensor.matmul(
                    ps_s[:, NQ : 2 * NQ],
                    lhsT=kt_sb[j][64:128, kt * P : (kt + 1) * P],
                    rhs=qt_sb[64:128, j, :],
                    start=True,
                    stop=True,
                )
                e = ep.tile([P, 2 * NQ], BF16_DT, tag="e", name="e_pair")
                nc.scalar.activation(e, ps_s, EXP)
                e_tiles[(j, kt)] = e

            def att_av(j, kt):
                e = e_tiles.pop((j, kt))
                if kt == 0:
                    po[(j, 0)] = psa.tile([P, QB, D + 1], FP32, tag="av", name=f"poA{j}")
                    po[(j, 1)] = psa.tile([P, QB, D + 1], FP32, tag="av", name=f"poB{j}")
                for h01 in range(2):
                    for qb in range(QB):
                        # start=True resets has_written for the WHOLE bank, so
                        # only the first region per bank may set it; the other
                        # regions overwrite on their first write (has_written
                        # cleared) and accumulate afterwards.
                        nc.tensor.matmul(
                            po[(j, h01)][:, qb, :],
                            lhsT=e[:, h01 * NQ + qb * P : h01 * NQ + (qb + 1) * P],
                            rhs=v_sb[:, kt, 2 * j + h01, :],
                            start=(kt == 0 and qb == 0),
                            stop=(kt == NKT - 1),
                            skip_group_check=True,
                        )

            def att_phase(j, fillers_of_kt):
                """Software-pipelined kt loop: PE order per iter is
                scores(kt) -> fillers -> attnV(kt-1), so exp(kt-1) has a full
                iteration of PE work to hide behind."""
                for kt in range(NKT):
                    att_scores(j, kt)
                    for f in fillers_of_kt(kt):
                        f()
                    if kt > 0:
                        att_av(j, kt - 1)
                att_av(j, NKT - 1)

            # ---- normalization + transpose back to [ac, q] -----------------
            def norm_qb(j, qb, pe_tr=False):
                # pe_tr (tail): normalize on Act + transpose on the (idle) PE
                # for minimum latency; otherwise DVE + DMA-xbar transpose.
                rcp = np_pool.tile([P, 2], FP32, tag="rcp", name="rcp")
                nrm = np_pool.tile([P, P], BF16_DT, tag="nrm", name="nrm")
                for h01 in range(2):
                    nc.vector.reciprocal(rcp[:, h01 : h01 + 1], po[(j, h01)][:, qb, D : D + 1])
                    if pe_tr:
                        nc.scalar.activation(
                            nrm[:, h01 * D : (h01 + 1) * D],
                            po[(j, h01)][:, qb, 0:D],
                            mybir.ActivationFunctionType.Copy,
                            scale=rcp[:, h01 : h01 + 1],
                        )
                    else:
                        nc.vector.tensor_scalar(
                            nrm[:, h01 * D : (h01 + 1) * D],
                            po[(j, h01)][:, qb, 0:D],
                            rcp[:, h01 : h01 + 1],
                            None,
                            AOP.mult,
                        )
                if pe_tr:
                    pt = pp.tile([P, P], BF16_DT, tag="pp", name="pt")
                    nc.tensor.transpose(pt, nrm, ident_sb)
                    nc.vector.tensor_copy(cat_sb[j][:, qb * P : (qb + 1) * P], pt)
                else:
                    nc.sync.dma_start_transpose(cat_sb[j][:, qb * P : (qb + 1) * P], nrm)

            # ---- incremental output projection: pair j's contribution to
            # out[qb block, half] accumulated into an fp32 SBUF accumulator,
            # so only pair 2's single matmul chain sits in the tail ----------
            out_acc = singles.tile([P, QB, 2, DG], FP32)
            ob_tiles = {}

            def outproj_partial(j, qb, half):
                ps = pp.tile([P, DG], FP32, tag="pp", name="ps_o")
                nc.tensor.matmul(
                    ps,
                    lhsT=cat_sb[j][:, qb * P : (qb + 1) * P],
                    rhs=wproj_sb[:, j, half * DG : (half + 1) * DG],
                    start=True,
                    stop=True,
                )
                acc = out_acc[:, qb, half, :]
                if j == 0:
                    nc.vector.tensor_copy(acc, ps)
                elif j == 1:
                    nc.vector.tensor_tensor(acc, acc, ps, AOP.add)
                else:
                    # final pair: add into a bf16 staging tile covering two q
                    # blocks; ship both in one DMA when the 4th quarter lands
                    grp = qb // 2
                    if grp not in ob_tiles:
                        ob_tiles[grp] = np_pool.tile(
                            [P, 2, 2, DG], BF16_DT, tag="ob", name="ob"
                        )
                    ob = ob_tiles[grp]
                    nc.vector.tensor_tensor(ob[:, qb % 2, half, :], acc, ps, AOP.add)
                    if qb % 2 == 1 and half == 1:
                        out_ap = out_d[:]
                        dst = bass.AP(
                            tensor=out_ap.tensor,
                            offset=out_ap.offset + grp * 2 * P * LATENT,
                            ap=[[LATENT, P], [P * LATENT, 2], [1, LATENT]],
                        )
                        src_ap = bass.AP(
                            tensor=ob.tensor,
                            offset=ob.offset,
                            ap=[list(a) for a in ob.ap[:1]]
                            + [[2 * DG, 2], [1, 2 * DG]],
                        )
                        nc.sync.dma_start(dst, src_ap)

            # ================ schedule ======================================
            # pre-phase: just enough for att(0) kts 0-7: K(0) ch 0-1 +
            # rope quarter 0 (Act-issued perm DMAs; Act idle here), V(0) 0-3
            for ch in range(2):
                kp_chunk(0, ch)
                vp(0, 2 * ch, "act")
                vp(0, 2 * ch + 1, "act")
            kp_rope(0, 0, dma_eng=nc.scalar)

            # att(0) filler table: kp chunks placed at data-chunk arrival
            # (d4 ~27us = iter 8), ropes feed kt quarter deadlines (quarter q
            # of the active pair needed by iter 8q), vp(0,k) due by iter k-1.
            KP0_SCHED = {0: (0, 2), 1: (0, 3), 3: (1, 0), 4: (1, 1),
                         6: (1, 2), 7: (1, 3), 8: (0, 4), 9: (0, 5),
                         11: (0, 6), 12: (0, 7), 14: (1, 4), 15: (1, 5),
                         17: (1, 6), 18: (1, 7)}
            ROPE0_SCHED = {2: (0, 1), 5: (1, 0), 10: (0, 2), 13: (0, 3),
                           16: (1, 1), 19: (1, 2), 22: (1, 3)}
            VP0_SCHED = {}
            for _i in range(6):
                VP0_SCHED[_i] = [(0, 4 + 2 * _i), (0, 5 + 2 * _i)]
            for _i in range(6, 8):
                VP0_SCHED[_i] = [(1, 2 * (_i - 6)), (1, 2 * (_i - 6) + 1)]
            for _i in range(8, 10):
                VP0_SCHED[_i] = [(0, 2 * (_i - 8) + 16), (0, 2 * (_i - 8) + 17)]
            for _i in range(10, 22):
                VP0_SCHED[_i] = [(0, _i + 10)]
            VP0_SCHED[20].append((1, 4))
            VP0_SCHED[21].append((1, 5))
            VP0_SCHED[22] = [(1, 6)]
            VP0_SCHED[23] = [(1, 7)]
            for _i in range(24, 32):
                VP0_SCHED[_i] = [(1, _i - 16)]      # vp(1, 8..15)

            def fill0(kt):
                fillers = []
                if kt in KP0_SCHED:
                    j_, c_ = KP0_SCHED[kt]
                    fillers.append(lambda j=j_, c=c_: kp_chunk(j, c))
                if kt in ROPE0_SCHED:
                    j_, q_ = ROPE0_SCHED[kt]
                    mul_eng = nc.gpsimd if (j_, q_) == (1, 3) else None
                    fillers.append(
                        lambda j=j_, q=q_, m=mul_eng: kp_rope(j, q, mul_eng=m)
                    )
                for pj, k in VP0_SCHED.get(kt, []):
                    fillers.append(lambda p_=pj, k_=k: vp(p_, k_))
                return fillers

            att_phase(0, fill0)
            for qb in range(QB):
                norm_qb(0, qb)

            # att(1): fill with K(2)+rope, V(1) kts 16-31, V(2) kts 0-9,
            # and pair-0's output-projection partials
            def fill1(kt):
                fillers = []
                if kt < 16 and kt % 2 == 0:
                    fillers.append(lambda c=kt // 2: kp_chunk(2, c))
                if kt in (3, 7, 11, 15):
                    q = (kt - 3) // 4
                    mul = nc.gpsimd if q == 3 else None
                    fillers.append(lambda q=q, m=mul: kp_rope(2, q, mul_eng=m))
                if kt < 16:
                    fillers.append(lambda k=kt + 16: vp(1, k))
                if 4 <= kt < 12:
                    fillers.append(
                        lambda qb=(kt - 4) // 2, h=kt % 2: outproj_partial(0, qb, h)
                    )
                if kt >= 22:
                    fillers.append(lambda k=kt - 22: vp(2, k))
                return fillers

            att_phase(1, fill1)
            for qb in range(QB):
                norm_qb(1, qb)

            # att(2): fill with V(2) kts 3-31 and pair-1's outproj partials
            def fill2(kt):
                fillers = []
                if kt < NKT - 10:
                    fillers.append(lambda k=kt + 10: vp(2, k))
                if 4 <= kt < 12:
                    fillers.append(
                        lambda qb=(kt - 4) // 2, h=kt % 2: outproj_partial(1, qb, h)
                    )
                return fillers

            att_phase(2, fill2)

            # tail: normalize pair 2 (PE transpose), outproj + output DMAs
            for qb in range(QB):
                norm_qb(2, qb, pe_tr=True)
                outproj_partial(2, qb, 0)
                outproj_partial(2, qb, 1)

    nc.finalize()
    return nc


_NC_CACHE = None


def _get_program():
    global _NC_CACHE
    if _NC_CACHE is None:
        _NC_CACHE = _build_program()
    return _NC_CACHE


def _host_inputs(latent, data, rope_q, rope_k, Wq, bq, Wkv, bkv, Wproj, bproj):
    assert not np.any(bq) and not np.any(bkv), "nonzero qkv biases unsupported"
    scale = D ** -0.5
    sign = np.concatenate([-np.ones(32, np.float32), np.ones(32, np.float32)])

    def rep(x):  # [64, n] -> [128, n], two head-copies
        return np.concatenate([x, x], axis=0).astype(BF16)

    sin_q, cos_q = rope_q[:, :D].T, rope_q[:, D:].T      # [64, 512]
    sin_k, cos_k = rope_k[:, :D].T, rope_k[:, D:].T      # [64, 4096]
    ropeq_r = np.stack([rep(cos_q), rep(sign[:, None] * sin_q)], axis=1)
    ropek_r = np.stack([rep(cos_k), rep(sign[:, None] * sin_k)], axis=1)

    in_maps = []
    for c in range(8):
        b, g = c // 2, c % 2
        sl = slice(g * DG, (g + 1) * DG)
        in_maps.append({
            "latentT": np.ascontiguousarray(latent[b].T).astype(BF16),
            "dataT": np.ascontiguousarray(data[b].T).astype(BF16),
            "wq": (Wq[:, sl] * scale).astype(BF16),
            "wk": Wkv[:, g * DG : (g + 1) * DG].astype(BF16),
            "wv": Wkv[:, LATENT + g * DG : LATENT + (g + 1) * DG].astype(BF16),
            "wproj": Wproj[sl, :].astype(BF16),
            "ropeq": ropeq_r, "ropek": ropek_r,
            "ident": np.eye(P, dtype=np.float32).astype(BF16),
        })
    return in_maps


def kernel(latent, data, rope_q, rope_k, Wq, bq, Wkv, bkv, Wproj, bproj,
           _trace=False):
    nc = _get_program()
    in_maps = _host_inputs(latent, data, rope_q, rope_k, Wq, bq, Wkv, bkv,
                           Wproj, bproj)
    res = run_bass_kernel_spmd(nc, in_maps, core_ids=list(range(8)),
                               trace=_trace)
    out = np.empty((B, NQ, LATENT), np.float32)
    for b in range(B):
        acc = (res.results[2 * b]["out"].astype(np.float32)
               + res.results[2 * b + 1]["out"].astype(np.float32))
        out[b] = acc + bproj[None, :]
    kernel.last_results = res
    return out
